# revision 2
# baseline (speedup 1.0000x reference)
"""Causal self-attention (B=2, N=2048, D=2048, H=16, hd=128) on 8 Trainium2
NeuronCores.

Strategy (tensor-parallel over heads, 2 heads/core), v3:
  - Host: transpose x / weights into contiguous per-partition layouts,
    build RoPE tables + 0/1 triangular mask const, slice w_qkv rows per
    head group.
  - Device, per core (same SPMD program, different input data), per batch
    the projection chunks and attention units are INTERLEAVED
    (unit (h,j) only needs chunks 0..j), so attention's ACT/DVE work
    hides under the projection's PE-bound matmuls:
      for j in 0..3:  phase_a_chunk(j);  phase_b_unit(h0,j);  phase_b_unit(h1,j)
    Phase A: qkvT projection (bf16 matmuls, outputs in [d, n] layout) + RoPE
             (DVE mul/add on psum pairs) -> stage tiles -> SBUF->SBUF DMA
             repack into per-head [128=hd, N] q/k tiles.
    Phase B (per unit): S.T = kh.T @ qh, with causal DIAGONAL tiles sliced
             to their valid query range (FD = 512-f0) -- no bias matmuls.
             exp on ACT over sliced ranges; causal band masked post-exp by
             DVE multiply with a 0/1 triangle const; row sums accumulated
             in bf16 on DVE + one ones-column matmul; reciprocal (DVE) +
             partition_broadcast (GPSIMD); O.T accumulated as vT.T @ P.T
             with diag tiles sliced.
    Per-(batch,head) AllToAll fired as soon as that head's units complete.
    Phase C: o_proj on the 2x256-row shard with w_o pre-cached in SBUF;
             all half-1 chains split at k=8 so only their upper halves
             depend on the last collective.
  - Host: reassemble [b0 rows 256c:256c+256 | b1 rows 256c:256c+256].
"""

import sys
import time

import ml_dtypes
import numpy as np

sys.path.insert(0, "/opt/trn_rl_repo")

import concourse.bacc as bacc  # noqa: E402
import concourse.bass as bass  # noqa: E402
import concourse.mybir as mybir  # noqa: E402
import concourse.tile as tile  # noqa: E402
from concourse import bass_utils  # noqa: E402

F32 = mybir.dt.float32
BF16 = mybir.dt.bfloat16

B, N, D = 2, 2048, 2048
H, HD = 16, 128
NC = 8
HPC = H // NC          # heads per core
BN = B * N             # 4096
NSH = BN // NC         # output rows per core
INNER = H * HD
ROPE_BASE = 10000.0

_CACHE = {}

LAST_EXEC_NS = None
LAST_RESULTS = None


def _build_program():
    nc = bacc.Bacc(
        "TRN2",
        target_bir_lowering=False,
        debug=False,
        enable_asserts=False,
        num_devices=NC,
    )
    xT = nc.dram_tensor("xT", [D, BN], BF16, kind="ExternalInput").ap()
    wqkT = nc.dram_tensor("wqkT", [D, 4 * HD], BF16, kind="ExternalInput").ap()
    wvf = nc.dram_tensor("wvf", [128, 16, HPC * HD], BF16, kind="ExternalInput").ap()
    wof = nc.dram_tensor("wof", [128, 16, D], BF16, kind="ExternalInput").ap()
    tabs = nc.dram_tensor("tabs", [4, HD, BN], BF16, kind="ExternalInput").ap()
    tri01 = nc.dram_tensor("tri01", [128, 1024], BF16, kind="ExternalInput").ap()
    # contiguous copy of the first x chunk / wqk: startup-critical DMAs run
    # contiguous instead of strided-gather; loaded in 256KB pieces so the
    # first matmul chain starts on the first piece.
    xf0 = nc.dram_tensor("xf0", [2, 128, 8, 512], BF16, kind="ExternalInput").ap()
    wqf = nc.dram_tensor("wqf", [2, 128, 8, 4 * HD], BF16, kind="ExternalInput").ap()
    out = nc.dram_tensor("out", [NSH, D], F32, kind="ExternalOutput").ap()
    a2a_in = [
        [
            nc.dram_tensor(f"a2a_in{b}_{h}", [NC, 128, 256], BF16).ap()
            for h in range(HPC)
        ]
        for b in range(B)
    ]
    a2a_out = [
        [
            nc.dram_tensor(f"a2a_out{b}_{h}", [NC, 128, 256], BF16).ap()
            for h in range(HPC)
        ]
        for b in range(B)
    ]

    MUL = mybir.AluOpType.mult
    ADD = mybir.AluOpType.add
    SUB = mybir.AluOpType.subtract
    EXP = mybir.ActivationFunctionType.Exp

    with tile.TileContext(nc, num_cores=NC) as tc:
        with (
            tc.tile_pool(name="const", bufs=1) as constp,
            tc.tile_pool(name="wqk", bufs=1) as wqkp,
            tc.tile_pool(name="wv", bufs=1) as wvp,
            tc.tile_pool(name="wo", bufs=1) as wop,
            tc.tile_pool(name="persist", bufs=1) as persist,
        ):
            wqk_sb = wqkp.tile([128, 16, 512], BF16, name="wqk_sb")
            wv_sb = wvp.tile([128, 16, 256], BF16, name="wv_sb")
            wo_sb = wop.tile([128, 16, D], BF16, name="wo_sb")
            tri_sb = constp.tile([128, 1024], BF16, name="tri_sb")
            ones_col = constp.tile([128, 1], BF16, name="ones_col")

            with (
                tc.tile_pool(name="xt", bufs=5) as xtp,
                tc.tile_pool(name="tab", bufs=2) as tabp,
                tc.tile_pool(name="rope", bufs=2) as ropep,
                tc.tile_pool(name="stage", bufs=3) as stagep,
                tc.tile_pool(name="pt", bufs=5) as ptp,
                tc.tile_pool(name="small", bufs=2) as smallp,
                tc.tile_pool(name="ots", bufs=3) as otsp,
                tc.tile_pool(name="rsc", bufs=2) as rscp,
                tc.tile_pool(name="bc", bufs=2) as bcp,
                tc.tile_pool(name="pst", bufs=2, space="PSUM") as pstp,
                tc.tile_pool(name="pov", bufs=2, space="PSUM") as povp,
                tc.tile_pool(name="psmall", bufs=2, space="PSUM") as psmallp,
            ):
                # startup loads, piecewise so the first matmul chain can
                # start as soon as its first 256KB lands.
                xh_first = []
                for half in range(2):
                    t = xtp.tile([128, 8, 512], BF16, tag="xt", name=f"xt_0_0_{half}")
                    for q in range(4):
                        nc.sync.dma_start(
                            out=t[:, 2 * q : 2 * q + 2, :],
                            in_=xf0[half][:, 2 * q : 2 * q + 2, :],
                        )
                    xh_first.append(t)
                    for q in range(4):
                        nc.sync.dma_start(
                            out=wqk_sb[:, 8 * half + 2 * q : 8 * half + 2 * q + 2, :],
                            in_=wqf[half][:, 2 * q : 2 * q + 2, :],
                        )
                nc.sync.dma_start(out=wv_sb[:, :, :], in_=wvf)
                nc.sync.dma_start(out=tri_sb[:, :], in_=tri01[:, :])
                nc.vector.memset(ones_col[:, :], 1.0)
                # first-use ptw tiles are partially read (then zeroed) by the
                # mask multiplies before exp ever wrote them: scrub SBUF
                # garbage (inf/nan bit patterns would make 0*x = nan).
                for q in range(5):
                    t = ptp.tile([128, 1024], BF16, tag="pt", name=f"ptz_{q}")
                    nc.vector.memset(t[:, :], 0.0)

                def load_x(b, j):
                    n0 = b * N + 512 * j
                    xh = []
                    for half in range(2):
                        t = xtp.tile(
                            [128, 8, 512], BF16, tag="xt", name=f"xt_{b}_{j}_{half}"
                        )
                        nc.sync.dma_start(
                            out=t[:, :, :],
                            in_=xT.rearrange("(k p) n -> p k n", p=128)[
                                :, 8 * half : 8 * half + 8, n0 : n0 + 512
                            ],
                        )
                        xh.append(t)
                    return xh

                def qk_pair(b, j, pair, xh, qh_sb, tabt):
                    pw = pstp.tile([128, 1024], F32, tag="pst", name=f"pw_{b}_{j}_{pair}")
                    psA = pw[:, 0:512]
                    psB = pw[:, 512:1024]
                    for mt, pst_ in ((pair, psA), (pair + 1, psB)):
                        for k in range(16):
                            nc.tensor.matmul(
                                pst_,
                                lhsT=(wqk_sb[:, k, 128 * mt : 128 * mt + 128]),
                                rhs=(xh[k // 8][:, k % 8, :]),
                                start=(k == 0),
                                stop=(k == 15),
                            )
                    ci = 0 if pair == 0 else 2
                    t1 = ropep.tile([128, 512], BF16, tag="t1", name=f"t1_{b}_{j}_{pair}")
                    t2 = ropep.tile([128, 512], BF16, tag="t2", name=f"t2_{b}_{j}_{pair}")
                    t3 = ropep.tile([128, 512], BF16, tag="t3", name=f"t3_{b}_{j}_{pair}")
                    t4 = ropep.tile([128, 512], BF16, tag="t4", name=f"t4_{b}_{j}_{pair}")
                    nc.vector.tensor_tensor(t1[:, :], psA, tabt[ci][:, :], MUL)
                    nc.vector.tensor_tensor(t2[:, :], psB, tabt[ci + 1][:, :], MUL)
                    nc.vector.tensor_tensor(t3[:, :], psB, tabt[ci][:, :], MUL)
                    nc.vector.tensor_tensor(t4[:, :], psA, tabt[ci + 1][:, :], MUL)
                    sl = stagep.tile([128, 512], BF16, tag="sl", name=f"sl_{b}_{j}_{pair}")
                    sh = stagep.tile([128, 512], BF16, tag="sh", name=f"sh_{b}_{j}_{pair}")
                    nc.vector.tensor_tensor(sl[:, :], t1[:, :], t2[:, :], SUB)
                    nc.vector.tensor_tensor(sh[:, :], t3[:, :], t4[:, :], ADD)
                    # repack: per-head [lo;hi] tiles for full-contract scores.
                    base = 0 if pair == 0 else 2
                    cs = slice(512 * j, 512 * (j + 1))
                    nc.sync.dma_start(out=qh_sb[0:64, base, cs], in_=sl[0:64, :])
                    nc.sync.dma_start(out=qh_sb[0:64, base + 1, cs], in_=sl[64:128, :])
                    nc.sync.dma_start(out=qh_sb[64:128, base, cs], in_=sh[0:64, :])
                    nc.sync.dma_start(out=qh_sb[64:128, base + 1, cs], in_=sh[64:128, :])

                def phase_a_chunk(b, j, qh_sb, vT_sb):
                    n0 = b * N + 512 * j
                    xh = xh_first if (b, j) == (0, 0) else load_x(b, j)
                    tabt = []
                    for ti in range(4):
                        tt = tabp.tile([128, 512], BF16, tag=f"tab{ti}", name=f"tab{ti}_{b}_{j}")
                        nc.sync.dma_start(out=tt[:, :], in_=tabs[ti, :, n0 : n0 + 512])
                        tabt.append(tt)
                    for pair in (0, 2):
                        qk_pair(b, j, pair, xh, qh_sb, tabt)
                    for mt in range(4):
                        pv = povp.tile([128, 256], F32, tag="pov", name=f"psV_{b}_{j}_{mt}")
                        for k in range(16):
                            nc.tensor.matmul(
                                pv[:, :],
                                lhsT=(xh[k // 8][:, k % 8, 128 * mt : 128 * mt + 128]),
                                rhs=(wv_sb[:, k, :]),
                                start=(k == 0),
                                stop=(k == 15),
                            )
                        nc.scalar.copy(vT_sb[:, 4 * j + mt, :], pv[:, :])

                def phase_b_unit(b, h, j, qh_sb, vT_sb):
                    nt = 4 * j + 4
                    nslab = nt // 2
                    ov = povp.tile([128, 512], F32, tag="pov", name=f"ov_{b}_{h}_{j}")
                    rs_c = rscp.tile([128, 512], BF16, tag="rsc", name=f"rsc_{b}_{h}_{j}")
                    issued = 0
                    for m in range(nslab):
                        # the two diagonal slabs pack (higher, lower) key
                        # tiles so the valid region stays contiguous-ish.
                        if m == nslab - 2:
                            pair = (4 * j + 1, 4 * j + 0)
                            diag = True
                        elif m == nslab - 1:
                            pair = (4 * j + 3, 4 * j + 2)
                            diag = True
                        else:
                            pair = (2 * m, 2 * m + 1)
                            diag = False
                        pw = pstp.tile(
                            [128, 1024], F32, tag="pst", name=f"stw_{b}_{h}_{j}_{m}"
                        )
                        ptw = ptp.tile(
                            [128, 1024], BF16, tag="pt", name=f"pt_{b}_{h}_{j}_{m}"
                        )
                        for half, t in enumerate(pair):
                            if diag:
                                f0 = 128 * (t - 4 * j)
                            else:
                                f0 = 0
                            nc.tensor.matmul(
                                pw[:, 512 * half + f0 : 512 * half + 512],
                                lhsT=(qh_sb[:, 2 + h, 128 * t : 128 * t + 128]),
                                rhs=(qh_sb[:, h, 512 * j + f0 : 512 * (j + 1)]),
                                start=True,
                                stop=True,
                            )
                        if diag:
                            lo = 128 if m == nslab - 2 else 384
                            nc.scalar.activation(ptw[:, lo:1024], pw[:, lo:1024], EXP)
                        else:
                            nc.scalar.activation(ptw[:, :], pw[:, :], EXP)
                        if diag:
                            # zero the fully-masked cols + the causal band
                            for half, t in enumerate(pair):
                                f0 = 128 * (t - 4 * j)
                                w = f0 + 128
                                nc.vector.tensor_tensor(
                                    ptw[:, 512 * half : 512 * half + w],
                                    ptw[:, 512 * half : 512 * half + w],
                                    tri_sb[:, 512 - f0 : 512 - f0 + w],
                                    MUL,
                                )
                        # bf16 row-sum accumulation over key tiles
                        if m == 0:
                            nc.vector.tensor_tensor(
                                rs_c[:, :], ptw[:, 0:512], ptw[:, 512:1024], ADD
                            )
                        else:
                            nc.vector.tensor_tensor(
                                rs_c[:, :], rs_c[:, :], ptw[:, 0:512], ADD
                            )
                            nc.vector.tensor_tensor(
                                rs_c[:, :], rs_c[:, :], ptw[:, 512:1024], ADD
                            )
                        for half, t in enumerate(pair):
                            f0 = 128 * (t - 4 * j) if diag else 0
                            issued += 1
                            nc.tensor.matmul(
                                ov[:, f0:512],
                                lhsT=(vT_sb[:, t, 128 * h : 128 * h + 128]),
                                rhs=(ptw[:, 512 * half + f0 : 512 * half + 512]),
                                start=(issued == 1),
                                stop=(issued == nt),
                            )
                    rsum = psmallp.tile(
                        [1, 512], F32, tag="rsum", name=f"rsum_{b}_{h}_{j}"
                    )
                    nc.tensor.matmul(
                        rsum[:, :],
                        lhsT=ones_col[:, :],
                        rhs=rs_c[:, :],
                        start=True,
                        stop=True,
                    )
                    rinv = smallp.tile([1, 512], F32, tag="rinv", name=f"rinv_{b}_{h}_{j}")
                    nc.vector.reciprocal_approx_fast(rinv[:, :], rsum[:, :])
                    binv = bcp.tile([128, 512], F32, tag="binv", name=f"binv_{b}_{h}_{j}")
                    nc.gpsimd.partition_broadcast(binv[:, :], rinv[:, :])
                    ot = otsp.tile([128, 512], BF16, tag="ot", name=f"ot_{b}_{h}_{j}")
                    nc.vector.tensor_tensor(ot[:, :], ov[:, :], binv[:, :], MUL)
                    nc.sync.dma_start(
                        out=a2a_in[b][h][2 * j, :, :], in_=ot[:, 0:256]
                    )
                    nc.sync.dma_start(
                        out=a2a_in[b][h][2 * j + 1, :, :], in_=ot[:, 256:512]
                    )

                def emit_cc(b, h):
                    nc.gpsimd.collective_compute(
                        "AllToAll",
                        mybir.AluOpType.bypass,
                        replica_groups=[list(range(NC))],
                        ins=[a2a_in[b][h].opt()],
                        outs=[a2a_out[b][h].opt()],
                    )

                for b in range(B):
                    qh_sb = persist.tile(
                        [128, 4, N], BF16, tag="qh", name=f"qh_b{b}"
                    )
                    vT_sb = persist.tile(
                        [128, 16, HPC * HD], BF16, tag="vT", name=f"vT_b{b}"
                    )
                    # projection chunks interleaved with the attention units
                    # they unblock: unit (h, j) needs only chunks 0..j.
                    for j in range(4):
                        phase_a_chunk(b, j, qh_sb, vT_sb)
                        for h in range(HPC):
                            phase_b_unit(b, h, j, qh_sb, vT_sb)
                    if b == 0:
                        # w_o cache fill rides under batch-0 compute
                        nc.sync.dma_start(out=wo_sb[:, :, :], in_=wof)
                    for h in range(HPC):
                        emit_cc(b, h)

            # ---------------- phase C: o_proj ------------------------------
            # opin k-tile order is (h, src) -> wof rows are host-permuted to
            # match.  All half-1 chains split at the k=8 boundary: the low
            # halves depend only on the (b1,h0) collective, so they fill the
            # window while the last collective completes.
            with (
                tc.tile_pool(name="opin", bufs=1) as opinp,
                tc.tile_pool(name="outs", bufs=4) as outsp,
                tc.tile_pool(name="pc", bufs=8, space="PSUM") as pcp,
            ):
                opins = []
                for half in range(B):
                    opin = opinp.tile([128, 16, 256], BF16, tag=f"opin{half}", name=f"opin{half}")
                    for h in range(HPC):
                        nc.sync.dma_start(
                            out=opin[:, 8 * h : 8 * h + 8, :],
                            in_=a2a_out[half][h].rearrange("r p n -> p r n"),
                        )
                    opins.append(opin)

                def c_chain(half, ns, dc, pc, k0, k1):
                    opin = opins[half]
                    for k in range(k0, k1):
                        nc.tensor.matmul(
                            pc[:, :],
                            lhsT=(opin[:, k, 128 * ns : 128 * ns + 128]),
                            rhs=(wo_sb[:, k, 512 * dc : 512 * (dc + 1)]),
                            start=(k == 0),
                            stop=(k == 15),
                        )

                def c_finish(half, ns, dc, pc, ost):
                    nc.scalar.copy(ost[:, 512 * dc : 512 * (dc + 1)], pc[:, :])
                    r0 = 256 * half + 128 * ns
                    nc.sync.dma_start(
                        out=out[r0 : r0 + 128, 512 * dc : 512 * (dc + 1)],
                        in_=ost[:, 512 * dc : 512 * (dc + 1)],
                    )

                osts = {}
                for ns in range(2):
                    osts[0, ns] = outsp.tile([128, D], F32, tag="outs", name=f"os_0_{ns}")
                    for dc in range(4):
                        pc = pcp.tile([128, 512], F32, tag="pc", name=f"pc_0_{dc}_{ns}")
                        c_chain(0, ns, dc, pc, 0, 16)
                        c_finish(0, ns, dc, pc, osts[0, ns])
                pcs = {}
                for ns in range(2):
                    osts[1, ns] = outsp.tile([128, D], F32, tag="outs", name=f"os_1_{ns}")
                    for dc in range(4):
                        pcs[ns, dc] = pcp.tile([128, 512], F32, tag="pc", name=f"pc_1_{dc}_{ns}")
                        c_chain(1, ns, dc, pcs[ns, dc], 0, 8)
                for ns in range(2):
                    for dc in range(4):
                        c_chain(1, ns, dc, pcs[ns, dc], 8, 16)
                        c_finish(1, ns, dc, pcs[ns, dc], osts[1, ns])
    nc.compile()
    return nc


def _host_prep(x, w_qkv, w_o):
    bf = ml_dtypes.bfloat16
    xT = np.ascontiguousarray(x.reshape(BN, D).T).astype(bf)
    # o_proj k-tile order on device is (h_local, src_core): head g lives at
    # slot 8*(g % 2) + g // 2.  wof is the [p, k, d] partition-major pack.
    woT_n = np.asarray(w_o).T.reshape(H, HD, D)
    perm = [2 * s + hl for hl in range(HPC) for s in range(NC)]
    woT = np.ascontiguousarray(woT_n[perm].reshape(INNER, D)).astype(bf)
    wof = np.ascontiguousarray(woT.reshape(16, 128, D).transpose(1, 0, 2))

    inv_freq = 1.0 / (ROPE_BASE ** (np.arange(0, HD, 2, dtype=np.float32) / HD))
    ang = np.arange(N, dtype=np.float32)[:, None] * inv_freq[None, :]
    cos_h = np.cos(ang).T.astype(np.float32)      # [64, N]
    sin_h = np.sin(ang).T.astype(np.float32)      # [64, N] (magnitude)
    # duplicated for the two heads packed per 128-row block
    cos2 = np.concatenate([cos_h, cos_h], axis=0)  # [128, N]
    sin2 = np.concatenate([sin_h, sin_h], axis=0)
    cos_f = np.tile(cos2, (1, B))
    sin_f = np.tile(sin2, (1, B))
    scale = np.float32(1.0 / np.sqrt(HD))
    tabs = np.ascontiguousarray(
        np.stack([cos_f * scale, sin_f * scale, cos_f, sin_f], axis=0)
    ).astype(bf)

    xf0 = np.ascontiguousarray(
        xT.reshape(16, 128, BN)[:, :, 0:512]
        .reshape(2, 8, 128, 512)
        .transpose(0, 2, 1, 3)
    )

    p = np.arange(128)[:, None]
    c = np.arange(1024)[None, :]
    # multiplicative causal mask: 1 where visible, 0 where masked
    tri01 = np.where(p <= c - 512, 1.0, 0.0).astype(bf)

    in_maps = []
    for core in range(NC):
        h0 = core * HPC
        rq = slice(h0 * HD, (h0 + HPC) * HD)
        rk = slice(INNER + h0 * HD, INNER + (h0 + HPC) * HD)
        rv = slice(2 * INNER + h0 * HD, 2 * INNER + (h0 + HPC) * HD)
        wq = w_qkv[rq].reshape(HPC, HD, D)
        wk = w_qkv[rk].reshape(HPC, HD, D)
        # row order per block: [h0_lo, h1_lo | h0_hi, h1_hi] for q then k
        wqkT = np.ascontiguousarray(
            np.concatenate(
                [wq[0, :64], wq[1, :64], wq[0, 64:], wq[1, 64:],
                 wk[0, :64], wk[1, :64], wk[0, 64:], wk[1, 64:]], axis=0
            ).T
        ).astype(bf)
        wvT = np.ascontiguousarray(w_qkv[rv].T).astype(bf)
        wvfc = np.ascontiguousarray(wvT.reshape(16, 128, HPC * HD).transpose(1, 0, 2))
        wqf = np.ascontiguousarray(
            wqkT.reshape(2, 8, 128, 4 * HD).transpose(0, 2, 1, 3)
        )
        in_maps.append(
            dict(xT=xT, wqkT=wqkT, wvf=wvfc, wof=wof, tabs=tabs, tri01=tri01,
                 xf0=xf0, wqf=wqf)
        )
    return in_maps


def kernel(x, w_qkv, w_o, n_heads=None, head_dim=None, trace=False):
    global LAST_EXEC_NS, LAST_RESULTS
    x = np.asarray(x, dtype=np.float32)
    w_qkv = np.asarray(w_qkv, dtype=np.float32)
    w_o = np.asarray(w_o, dtype=np.float32)

    if "nc" not in _CACHE:
        _CACHE["nc"] = _build_program()
    nc = _CACHE["nc"]

    in_maps = _host_prep(x, w_qkv, w_o)
    res = None
    last_exc = None
    for attempt in range(4):
        try:
            res = bass_utils.run_bass_kernel_spmd(
                nc, in_maps, core_ids=list(range(NC)), trace=trace
            )
            break
        except Exception as e:  # transient compile_and_load / exec flakiness
            last_exc = e
            print(f"kernel attempt {attempt} failed: {e}", file=sys.stderr)
            time.sleep(5)
    if res is None:
        raise last_exc
    LAST_EXEC_NS = res.exec_time_ns
    LAST_RESULTS = res
    # core c returns [512, D]: rows 0:256 = batch0 rows 256c:256c+256,
    # rows 256:512 = batch1 rows 256c:256c+256.
    full = np.empty((B, N, D), dtype=np.float32)
    for c in range(NC):
        shard = res.results[c]["out"]
        full[0, 256 * c : 256 * c + 256] = shard[0:256]
        full[1, 256 * c : 256 * c + 256] = shard[256:512]
    return full


# revision 8
# speedup vs baseline: 1.1426x; 1.1426x over previous
"""Causal self-attention (B=2, N=2048, D=2048, H=16, hd=128) on 8 Trainium2
NeuronCores.

Strategy (tensor-parallel over heads, 2 heads/core), v3:
  - Host: transpose x / weights into contiguous per-partition layouts,
    build RoPE tables + 0/1 triangular mask const, slice w_qkv rows per
    head group.
  - Device, per core (same SPMD program, different input data), per batch
    the projection chunks and attention units are INTERLEAVED
    (unit (h,j) only needs chunks 0..j), so attention's ACT/DVE work
    hides under the projection's PE-bound matmuls:
      for j in 0..3:  phase_a_chunk(j);  phase_b_unit(h0,j);  phase_b_unit(h1,j)
    Phase A: qkvT projection (bf16 matmuls, outputs in [d, n] layout) + RoPE
             (DVE mul/add on psum pairs) -> stage tiles -> SBUF->SBUF DMA
             repack into per-head [128=hd, N] q/k tiles.
    Phase B (per unit): S.T = kh.T @ qh, with causal DIAGONAL tiles sliced
             to their valid query range (FD = 512-f0) -- no bias matmuls.
             exp on ACT over sliced ranges; causal band masked post-exp by
             DVE multiply with a 0/1 triangle const; row sums accumulated
             in bf16 on DVE + one ones-column matmul; reciprocal (DVE) +
             partition_broadcast (GPSIMD); O.T accumulated as vT.T @ P.T
             with diag tiles sliced.
    Per-(batch,head) AllToAll fired as soon as that head's units complete.
    Phase C: o_proj on the 2x256-row shard with w_o pre-cached in SBUF;
             all half-1 chains split at k=8 so only their upper halves
             depend on the last collective.
  - Host: reassemble [b0 rows 256c:256c+256 | b1 rows 256c:256c+256].
"""

import sys
import time

import ml_dtypes
import numpy as np

sys.path.insert(0, "/opt/trn_rl_repo")

import concourse.bacc as bacc  # noqa: E402
import concourse.bass as bass  # noqa: E402
import concourse.mybir as mybir  # noqa: E402
import concourse.tile as tile  # noqa: E402
from concourse import bass_utils  # noqa: E402

F32 = mybir.dt.float32
BF16 = mybir.dt.bfloat16

B, N, D = 2, 2048, 2048
H, HD = 16, 128
NC = 8
HPC = H // NC          # heads per core
BN = B * N             # 4096
NSH = BN // NC         # output rows per core
INNER = H * HD
ROPE_BASE = 10000.0

_CACHE = {}

LAST_EXEC_NS = None
LAST_RESULTS = None


def _build_program():
    nc = bacc.Bacc(
        "TRN2",
        target_bir_lowering=False,
        debug=False,
        enable_asserts=False,
        num_devices=NC,
    )
    xT = nc.dram_tensor("xT", [D, BN], BF16, kind="ExternalInput").ap()
    wqkT = nc.dram_tensor("wqkT", [D, 4 * HD], BF16, kind="ExternalInput").ap()
    wvf = nc.dram_tensor("wvf", [128, 16, HPC * HD], BF16, kind="ExternalInput").ap()
    wof = nc.dram_tensor("wof", [128, 16, D], BF16, kind="ExternalInput").ap()
    tabs = nc.dram_tensor("tabs", [4, HD, BN], BF16, kind="ExternalInput").ap()
    tri01 = nc.dram_tensor("tri01", [128, 1024], BF16, kind="ExternalInput").ap()
    # contiguous copy of the first x chunk / wqk: startup-critical DMAs run
    # contiguous instead of strided-gather; loaded in 256KB pieces so the
    # first matmul chain starts on the first piece.
    xf0 = nc.dram_tensor("xf0", [2, 128, 8, 512], BF16, kind="ExternalInput").ap()
    wqf = nc.dram_tensor("wqf", [2, 128, 8, 4 * HD], BF16, kind="ExternalInput").ap()
    out = nc.dram_tensor("out", [NSH, D], F32, kind="ExternalOutput").ap()
    a2a_in = [
        [
            nc.dram_tensor(f"a2a_in{b}_{h}", [NC, 128, 256], BF16).ap()
            for h in range(HPC)
        ]
        for b in range(B)
    ]
    a2a_out = [
        [
            nc.dram_tensor(f"a2a_out{b}_{h}", [NC, 128, 256], BF16).ap()
            for h in range(HPC)
        ]
        for b in range(B)
    ]

    MUL = mybir.AluOpType.mult
    ADD = mybir.AluOpType.add
    SUB = mybir.AluOpType.subtract
    EXP = mybir.ActivationFunctionType.Exp

    with tile.TileContext(nc, num_cores=NC) as tc:
        with (
            tc.tile_pool(name="const", bufs=1) as constp,
            tc.tile_pool(name="wqk", bufs=1) as wqkp,
            tc.tile_pool(name="wv", bufs=1) as wvp,
            tc.tile_pool(name="wo", bufs=1) as wop,
            tc.tile_pool(name="persist", bufs=1) as persist,
        ):
            wqk_sb = wqkp.tile([128, 16, 512], BF16, name="wqk_sb")
            wv_sb = wvp.tile([128, 16, 256], BF16, name="wv_sb")
            wo_sb = wop.tile([128, 16, D], BF16, name="wo_sb")
            tri_sb = constp.tile([128, 1024], BF16, name="tri_sb")
            ones_col = constp.tile([128, 1], BF16, name="ones_col")

            with (
                tc.tile_pool(name="xt", bufs=5) as xtp,
                tc.tile_pool(name="tab", bufs=2) as tabp,
                tc.tile_pool(name="rope", bufs=2) as ropep,
                tc.tile_pool(name="stage", bufs=3) as stagep,
                tc.tile_pool(name="pt", bufs=5) as ptp,
                tc.tile_pool(name="small", bufs=2) as smallp,
                tc.tile_pool(name="ots", bufs=3) as otsp,
                tc.tile_pool(name="rsc", bufs=2) as rscp,
                tc.tile_pool(name="bc", bufs=2) as bcp,
                tc.tile_pool(name="pst", bufs=2, space="PSUM") as pstp,
                tc.tile_pool(name="pov", bufs=2, space="PSUM") as povp,
                tc.tile_pool(name="psmall", bufs=2, space="PSUM") as psmallp,
            ):
                # startup loads, piecewise so the first matmul chain can
                # start as soon as its first 256KB lands.
                xh_first = []
                for half in range(2):
                    t = xtp.tile([128, 8, 512], BF16, tag="xt", name=f"xt_0_0_{half}")
                    for q in range(4):
                        nc.sync.dma_start(
                            out=t[:, 2 * q : 2 * q + 2, :],
                            in_=xf0[half][:, 2 * q : 2 * q + 2, :],
                        )
                    xh_first.append(t)
                    for q in range(4):
                        nc.sync.dma_start(
                            out=wqk_sb[:, 8 * half + 2 * q : 8 * half + 2 * q + 2, :],
                            in_=wqf[half][:, 2 * q : 2 * q + 2, :],
                        )
                nc.sync.dma_start(out=wv_sb[:, :, :], in_=wvf)
                nc.sync.dma_start(out=tri_sb[:, :], in_=tri01[:, :])
                nc.vector.memset(ones_col[:, :], 1.0)
                # first-use ptw tiles are partially read (then zeroed) by the
                # mask multiplies before exp ever wrote them: scrub SBUF
                # garbage (inf/nan bit patterns would make 0*x = nan).
                for q in range(5):
                    t = ptp.tile([128, 1024], BF16, tag="pt", name=f"ptz_{q}")
                    nc.vector.memset(t[:, :], 0.0)

                def load_x(b, j):
                    n0 = b * N + 512 * j
                    xh = []
                    for half in range(2):
                        t = xtp.tile(
                            [128, 8, 512], BF16, tag="xt", name=f"xt_{b}_{j}_{half}"
                        )
                        nc.sync.dma_start(
                            out=t[:, :, :],
                            in_=xT.rearrange("(k p) n -> p k n", p=128)[
                                :, 8 * half : 8 * half + 8, n0 : n0 + 512
                            ],
                        )
                        xh.append(t)
                    return xh

                def load_tabs(b, j):
                    n0 = b * N + 512 * j
                    tabt = []
                    for ti in range(4):
                        tt = tabp.tile([128, 512], BF16, tag=f"tab{ti}", name=f"tab{ti}_{b}_{j}")
                        nc.sync.dma_start(out=tt[:, :], in_=tabs[ti, :, n0 : n0 + 512])
                        tabt.append(tt)
                    return tabt

                def qk_pair(b, j, pair, xh, qh_sb, tabt):
                    pw = pstp.tile([128, 1024], F32, tag="pst", name=f"pw_{b}_{j}_{pair}")
                    psA = pw[:, 0:512]
                    psB = pw[:, 512:1024]
                    for mt, pst_ in ((pair, psA), (pair + 1, psB)):
                        for k in range(16):
                            nc.tensor.matmul(
                                pst_,
                                lhsT=(wqk_sb[:, k, 128 * mt : 128 * mt + 128]),
                                rhs=(xh[k // 8][:, k % 8, :]),
                                start=(k == 0),
                                stop=(k == 15),
                            )
                    ci = 0 if pair == 0 else 2
                    t1 = ropep.tile([128, 512], BF16, tag="t1", name=f"t1_{b}_{j}_{pair}")
                    t2 = ropep.tile([128, 512], BF16, tag="t2", name=f"t2_{b}_{j}_{pair}")
                    t3 = ropep.tile([128, 512], BF16, tag="t3", name=f"t3_{b}_{j}_{pair}")
                    t4 = ropep.tile([128, 512], BF16, tag="t4", name=f"t4_{b}_{j}_{pair}")
                    nc.vector.tensor_tensor(t1[:, :], psA, tabt[ci][:, :], MUL)
                    nc.vector.tensor_tensor(t2[:, :], psB, tabt[ci + 1][:, :], MUL)
                    nc.vector.tensor_tensor(t3[:, :], psB, tabt[ci][:, :], MUL)
                    nc.vector.tensor_tensor(t4[:, :], psA, tabt[ci + 1][:, :], MUL)
                    sl = stagep.tile([128, 512], BF16, tag="sl", name=f"sl_{b}_{j}_{pair}")
                    sh = stagep.tile([128, 512], BF16, tag="sh", name=f"sh_{b}_{j}_{pair}")
                    nc.vector.tensor_tensor(sl[:, :], t1[:, :], t2[:, :], SUB)
                    nc.vector.tensor_tensor(sh[:, :], t3[:, :], t4[:, :], ADD)
                    # repack: per-head [lo;hi] tiles for full-contract scores.
                    base = 0 if pair == 0 else 2
                    cs = slice(512 * j, 512 * (j + 1))
                    nc.sync.dma_start(out=qh_sb[0:64, base, cs], in_=sl[0:64, :])
                    nc.sync.dma_start(out=qh_sb[0:64, base + 1, cs], in_=sl[64:128, :])
                    nc.sync.dma_start(out=qh_sb[64:128, base, cs], in_=sh[0:64, :])
                    nc.sync.dma_start(out=qh_sb[64:128, base + 1, cs], in_=sh[64:128, :])

                def phase_a_chunk(b, j, qh_sb, vT_sb, xh, tabt):
                    for pair in (0, 2):
                        qk_pair(b, j, pair, xh, qh_sb, tabt)
                    for mt in range(4):
                        pv = povp.tile([128, 256], F32, tag="pov", name=f"psV_{b}_{j}_{mt}")
                        for k in range(16):
                            nc.tensor.matmul(
                                pv[:, :],
                                lhsT=(xh[k // 8][:, k % 8, 128 * mt : 128 * mt + 128]),
                                rhs=(wv_sb[:, k, :]),
                                start=(k == 0),
                                stop=(k == 15),
                            )
                        nc.scalar.copy(vT_sb[:, 4 * j + mt, :], pv[:, :])

                def phase_b_unit(b, h, j, qh_sb, vT_sb):
                    nt = 4 * j + 4
                    nslab = nt // 2
                    ov = povp.tile([128, 512], F32, tag="pov", name=f"ov_{b}_{h}_{j}")
                    rs_c = rscp.tile([128, 512], BF16, tag="rsc", name=f"rsc_{b}_{h}_{j}")
                    issued = 0
                    for m in range(nslab):
                        # the two diagonal slabs pack (higher, lower) key
                        # tiles so the valid region stays contiguous-ish.
                        if m == nslab - 2:
                            pair = (4 * j + 1, 4 * j + 0)
                            diag = True
                        elif m == nslab - 1:
                            pair = (4 * j + 3, 4 * j + 2)
                            diag = True
                        else:
                            pair = (2 * m, 2 * m + 1)
                            diag = False
                        pw = pstp.tile(
                            [128, 1024], F32, tag="pst", name=f"stw_{b}_{h}_{j}_{m}"
                        )
                        ptw = ptp.tile(
                            [128, 1024], BF16, tag="pt", name=f"pt_{b}_{h}_{j}_{m}"
                        )
                        for half, t in enumerate(pair):
                            if diag:
                                f0 = 128 * (t - 4 * j)
                            else:
                                f0 = 0
                            nc.tensor.matmul(
                                pw[:, 512 * half + f0 : 512 * half + 512],
                                lhsT=(qh_sb[:, 2 + h, 128 * t : 128 * t + 128]),
                                rhs=(qh_sb[:, h, 512 * j + f0 : 512 * (j + 1)]),
                                start=True,
                                stop=True,
                            )
                        if diag:
                            lo = 128 if m == nslab - 2 else 384
                            nc.scalar.activation(ptw[:, lo:1024], pw[:, lo:1024], EXP)
                        else:
                            nc.scalar.activation(ptw[:, :], pw[:, :], EXP)
                        if diag:
                            # zero the fully-masked cols + the causal band
                            for half, t in enumerate(pair):
                                f0 = 128 * (t - 4 * j)
                                w = f0 + 128
                                nc.vector.tensor_tensor(
                                    ptw[:, 512 * half : 512 * half + w],
                                    ptw[:, 512 * half : 512 * half + w],
                                    tri_sb[:, 512 - f0 : 512 - f0 + w],
                                    MUL,
                                )
                        # bf16 row-sum accumulation over key tiles
                        if m == 0:
                            nc.vector.tensor_tensor(
                                rs_c[:, :], ptw[:, 0:512], ptw[:, 512:1024], ADD
                            )
                        else:
                            nc.vector.tensor_tensor(
                                rs_c[:, :], rs_c[:, :], ptw[:, 0:512], ADD
                            )
                            nc.vector.tensor_tensor(
                                rs_c[:, :], rs_c[:, :], ptw[:, 512:1024], ADD
                            )
                        for half, t in enumerate(pair):
                            f0 = 128 * (t - 4 * j) if diag else 0
                            issued += 1
                            nc.tensor.matmul(
                                ov[:, f0:512],
                                lhsT=(vT_sb[:, t, 128 * h : 128 * h + 128]),
                                rhs=(ptw[:, 512 * half + f0 : 512 * half + 512]),
                                start=(issued == 1),
                                stop=(issued == nt),
                            )
                    rsum = psmallp.tile(
                        [1, 512], F32, tag="rsum", name=f"rsum_{b}_{h}_{j}"
                    )
                    nc.tensor.matmul(
                        rsum[:, :],
                        lhsT=ones_col[:, :],
                        rhs=rs_c[:, :],
                        start=True,
                        stop=True,
                    )
                    rinv = smallp.tile([1, 512], F32, tag="rinv", name=f"rinv_{b}_{h}_{j}")
                    nc.vector.reciprocal_approx_fast(rinv[:, :], rsum[:, :])
                    binv = bcp.tile([128, 512], F32, tag="binv", name=f"binv_{b}_{h}_{j}")
                    nc.gpsimd.partition_broadcast(binv[:, :], rinv[:, :])
                    ot = otsp.tile([128, 512], BF16, tag="ot", name=f"ot_{b}_{h}_{j}")
                    nc.vector.tensor_tensor(ot[:, :], ov[:, :], binv[:, :], MUL)
                    # a2a writes ride the (idle) gpsimd queue: their wait
                    # conditions must not block the sync queue's x prefetches.
                    nc.gpsimd.dma_start(
                        out=a2a_in[b][h][2 * j, :, :], in_=ot[:, 0:256]
                    )
                    nc.gpsimd.dma_start(
                        out=a2a_in[b][h][2 * j + 1, :, :], in_=ot[:, 256:512]
                    )

                def emit_cc(b, h):
                    nc.gpsimd.collective_compute(
                        "AllToAll",
                        mybir.AluOpType.bypass,
                        replica_groups=[list(range(NC))],
                        ins=[a2a_in[b][h].opt()],
                        outs=[a2a_out[b][h].opt()],
                    )

                # chunk (0,0) inputs are the startup piece loads
                pre_x = {(0, 0): xh_first}
                pre_tab = {(0, 0): load_tabs(0, 0)}
                for b in range(B):
                    qh_sb = persist.tile(
                        [128, 4, N], BF16, tag="qh", name=f"qh_b{b}"
                    )
                    vT_sb = persist.tile(
                        [128, 16, HPC * HD], BF16, tag="vT", name=f"vT_b{b}"
                    )
                    # projection chunks interleaved with the attention units
                    # they unblock: unit (h, j) needs only chunks 0..j.
                    # x/tabs for the NEXT chunk are prefetched first each
                    # iteration so no compute-gated DMA sits ahead of them
                    # in the sync queue.
                    for j in range(4):
                        nxt = (b, j + 1) if j < 3 else (b + 1, 0)
                        if nxt[0] < B:
                            pre_x[nxt] = load_x(*nxt)
                            pre_tab[nxt] = load_tabs(*nxt)
                        phase_a_chunk(
                            b, j, qh_sb, vT_sb,
                            pre_x.pop((b, j)), pre_tab.pop((b, j)),
                        )
                        for h in range(HPC):
                            phase_b_unit(b, h, j, qh_sb, vT_sb)
                            # each head's collective fires as soon as its
                            # last unit is done
                            if j == 3:
                                emit_cc(b, h)
                        if b == 0 and j == 1:
                            # w_o cache fill rides under batch-0 compute,
                            # clear of the batch-end collectives
                            nc.sync.dma_start(out=wo_sb[:, :, :], in_=wof)

            # ---------------- phase C: o_proj ------------------------------
            # opin k-tile order is (h, src) -> wof rows are host-permuted to
            # match.  All half-1 chains split at the k=8 boundary: the low
            # halves depend only on the (b1,h0) collective, so they fill the
            # window while the last collective completes.
            with (
                tc.tile_pool(name="opin", bufs=1) as opinp,
                tc.tile_pool(name="outs", bufs=4) as outsp,
                tc.tile_pool(name="pc", bufs=8, space="PSUM") as pcp,
            ):
                opins = []
                for half in range(B):
                    opin = opinp.tile([128, 16, 256], BF16, tag=f"opin{half}", name=f"opin{half}")
                    for h in range(HPC):
                        nc.sync.dma_start(
                            out=opin[:, 8 * h : 8 * h + 8, :],
                            in_=a2a_out[half][h].rearrange("r p n -> p r n"),
                        )
                    opins.append(opin)

                def c_chain(half, ns, dc, pc, k0, k1):
                    opin = opins[half]
                    for k in range(k0, k1):
                        nc.tensor.matmul(
                            pc[:, :],
                            lhsT=(opin[:, k, 128 * ns : 128 * ns + 128]),
                            rhs=(wo_sb[:, k, 512 * dc : 512 * (dc + 1)]),
                            start=(k == 0),
                            stop=(k == 15),
                        )

                def c_finish(half, ns, dc, pc, ost):
                    nc.scalar.copy(ost[:, 512 * dc : 512 * (dc + 1)], pc[:, :])
                    r0 = 256 * half + 128 * ns
                    # out DMAs on the scalar queue: the sync queue's head is
                    # the opin[1] loads waiting on the last collectives.
                    nc.scalar.dma_start(
                        out=out[r0 : r0 + 128, 512 * dc : 512 * (dc + 1)],
                        in_=ost[:, 512 * dc : 512 * (dc + 1)],
                    )

                osts = {}
                for ns in range(2):
                    osts[0, ns] = outsp.tile([128, D], F32, tag="outs", name=f"os_0_{ns}")
                    for dc in range(4):
                        pc = pcp.tile([128, 512], F32, tag="pc", name=f"pc_0_{dc}_{ns}")
                        c_chain(0, ns, dc, pc, 0, 16)
                        c_finish(0, ns, dc, pc, osts[0, ns])
                pcs = {}
                for ns in range(2):
                    osts[1, ns] = outsp.tile([128, D], F32, tag="outs", name=f"os_1_{ns}")
                    for dc in range(4):
                        pcs[ns, dc] = pcp.tile([128, 512], F32, tag="pc", name=f"pc_1_{dc}_{ns}")
                        c_chain(1, ns, dc, pcs[ns, dc], 0, 8)
                for ns in range(2):
                    for dc in range(4):
                        c_chain(1, ns, dc, pcs[ns, dc], 8, 16)
                        c_finish(1, ns, dc, pcs[ns, dc], osts[1, ns])
    nc.compile()
    return nc


def _host_prep(x, w_qkv, w_o):
    bf = ml_dtypes.bfloat16
    xT = np.ascontiguousarray(x.reshape(BN, D).T).astype(bf)
    # o_proj k-tile order on device is (h_local, src_core): head g lives at
    # slot 8*(g % 2) + g // 2.  wof is the [p, k, d] partition-major pack.
    woT_n = np.asarray(w_o).T.reshape(H, HD, D)
    perm = [2 * s + hl for hl in range(HPC) for s in range(NC)]
    woT = np.ascontiguousarray(woT_n[perm].reshape(INNER, D)).astype(bf)
    wof = np.ascontiguousarray(woT.reshape(16, 128, D).transpose(1, 0, 2))

    inv_freq = 1.0 / (ROPE_BASE ** (np.arange(0, HD, 2, dtype=np.float32) / HD))
    ang = np.arange(N, dtype=np.float32)[:, None] * inv_freq[None, :]
    cos_h = np.cos(ang).T.astype(np.float32)      # [64, N]
    sin_h = np.sin(ang).T.astype(np.float32)      # [64, N] (magnitude)
    # duplicated for the two heads packed per 128-row block
    cos2 = np.concatenate([cos_h, cos_h], axis=0)  # [128, N]
    sin2 = np.concatenate([sin_h, sin_h], axis=0)
    cos_f = np.tile(cos2, (1, B))
    sin_f = np.tile(sin2, (1, B))
    scale = np.float32(1.0 / np.sqrt(HD))
    tabs = np.ascontiguousarray(
        np.stack([cos_f * scale, sin_f * scale, cos_f, sin_f], axis=0)
    ).astype(bf)

    xf0 = np.ascontiguousarray(
        xT.reshape(16, 128, BN)[:, :, 0:512]
        .reshape(2, 8, 128, 512)
        .transpose(0, 2, 1, 3)
    )

    p = np.arange(128)[:, None]
    c = np.arange(1024)[None, :]
    # multiplicative causal mask: 1 where visible, 0 where masked
    tri01 = np.where(p <= c - 512, 1.0, 0.0).astype(bf)

    in_maps = []
    for core in range(NC):
        h0 = core * HPC
        rq = slice(h0 * HD, (h0 + HPC) * HD)
        rk = slice(INNER + h0 * HD, INNER + (h0 + HPC) * HD)
        rv = slice(2 * INNER + h0 * HD, 2 * INNER + (h0 + HPC) * HD)
        wq = w_qkv[rq].reshape(HPC, HD, D)
        wk = w_qkv[rk].reshape(HPC, HD, D)
        # row order per block: [h0_lo, h1_lo | h0_hi, h1_hi] for q then k
        wqkT = np.ascontiguousarray(
            np.concatenate(
                [wq[0, :64], wq[1, :64], wq[0, 64:], wq[1, 64:],
                 wk[0, :64], wk[1, :64], wk[0, 64:], wk[1, 64:]], axis=0
            ).T
        ).astype(bf)
        wvT = np.ascontiguousarray(w_qkv[rv].T).astype(bf)
        wvfc = np.ascontiguousarray(wvT.reshape(16, 128, HPC * HD).transpose(1, 0, 2))
        wqf = np.ascontiguousarray(
            wqkT.reshape(2, 8, 128, 4 * HD).transpose(0, 2, 1, 3)
        )
        in_maps.append(
            dict(xT=xT, wqkT=wqkT, wvf=wvfc, wof=wof, tabs=tabs, tri01=tri01,
                 xf0=xf0, wqf=wqf)
        )
    return in_maps


def kernel(x, w_qkv, w_o, n_heads=None, head_dim=None, trace=False):
    global LAST_EXEC_NS, LAST_RESULTS
    x = np.asarray(x, dtype=np.float32)
    w_qkv = np.asarray(w_qkv, dtype=np.float32)
    w_o = np.asarray(w_o, dtype=np.float32)

    if "nc" not in _CACHE:
        _CACHE["nc"] = _build_program()
    nc = _CACHE["nc"]

    in_maps = _host_prep(x, w_qkv, w_o)
    res = None
    last_exc = None
    for attempt in range(4):
        try:
            res = bass_utils.run_bass_kernel_spmd(
                nc, in_maps, core_ids=list(range(NC)), trace=trace
            )
            break
        except Exception as e:  # transient compile_and_load / exec flakiness
            last_exc = e
            print(f"kernel attempt {attempt} failed: {e}", file=sys.stderr)
            time.sleep(5)
    if res is None:
        raise last_exc
    LAST_EXEC_NS = res.exec_time_ns
    LAST_RESULTS = res
    # core c returns [512, D]: rows 0:256 = batch0 rows 256c:256c+256,
    # rows 256:512 = batch1 rows 256c:256c+256.
    full = np.empty((B, N, D), dtype=np.float32)
    for c in range(NC):
        shard = res.results[c]["out"]
        full[0, 256 * c : 256 * c + 256] = shard[0:256]
        full[1, 256 * c : 256 * c + 256] = shard[256:512]
    return full


# revision 10
# speedup vs baseline: 1.1863x; 1.0382x over previous
"""Causal self-attention (B=2, N=2048, D=2048, H=16, hd=128) on 8 Trainium2
NeuronCores.

Strategy (tensor-parallel over heads, 2 heads/core), v3:
  - Host: transpose x / weights into contiguous per-partition layouts,
    build RoPE tables + 0/1 triangular mask const, slice w_qkv rows per
    head group.
  - Device, per core (same SPMD program, different input data), per batch
    the projection chunks and attention units are INTERLEAVED
    (unit (h,j) only needs chunks 0..j), so attention's ACT/DVE work
    hides under the projection's PE-bound matmuls:
      for j in 0..3:  phase_a_chunk(j);  phase_b_unit(h0,j);  phase_b_unit(h1,j)
    Phase A: qkvT projection (bf16 matmuls, outputs in [d, n] layout) + RoPE
             (DVE mul/add on psum pairs) -> stage tiles -> SBUF->SBUF DMA
             repack into per-head [128=hd, N] q/k tiles.
    Phase B (per unit): S.T = kh.T @ qh, with causal DIAGONAL tiles sliced
             to their valid query range (FD = 512-f0) -- no bias matmuls.
             exp on ACT over sliced ranges; causal band masked post-exp by
             DVE multiply with a 0/1 triangle const; row sums accumulated
             in bf16 on DVE + one ones-column matmul; reciprocal (DVE) +
             partition_broadcast (GPSIMD); O.T accumulated as vT.T @ P.T
             with diag tiles sliced.
    Per-(batch,head) AllToAll fired as soon as that head's units complete.
    Phase C: o_proj on the 2x256-row shard with w_o pre-cached in SBUF;
             all half-1 chains split at k=8 so only their upper halves
             depend on the last collective.
  - Host: reassemble [b0 rows 256c:256c+256 | b1 rows 256c:256c+256].
"""

import sys
import time

import ml_dtypes
import numpy as np

sys.path.insert(0, "/opt/trn_rl_repo")

import concourse.bacc as bacc  # noqa: E402
import concourse.bass as bass  # noqa: E402
import concourse.mybir as mybir  # noqa: E402
import concourse.tile as tile  # noqa: E402
from concourse import bass_utils  # noqa: E402

F32 = mybir.dt.float32
BF16 = mybir.dt.bfloat16

B, N, D = 2, 2048, 2048
H, HD = 16, 128
NC = 8
HPC = H // NC          # heads per core
BN = B * N             # 4096
NSH = BN // NC         # output rows per core
INNER = H * HD
ROPE_BASE = 10000.0

_CACHE = {}

LAST_EXEC_NS = None
LAST_RESULTS = None


def _build_program():
    nc = bacc.Bacc(
        "TRN2",
        target_bir_lowering=False,
        debug=False,
        enable_asserts=False,
        num_devices=NC,
    )
    xT = nc.dram_tensor("xT", [D, BN], BF16, kind="ExternalInput").ap()
    wqkT = nc.dram_tensor("wqkT", [D, 4 * HD], BF16, kind="ExternalInput").ap()
    wvf = nc.dram_tensor("wvf", [128, 16, HPC * HD], BF16, kind="ExternalInput").ap()
    wof = nc.dram_tensor("wof", [128, 16, D], BF16, kind="ExternalInput").ap()
    tabs = nc.dram_tensor("tabs", [4, HD, BN], BF16, kind="ExternalInput").ap()
    tri01 = nc.dram_tensor("tri01", [128, 1024], BF16, kind="ExternalInput").ap()
    # contiguous copy of the first x chunk / wqk: startup-critical DMAs run
    # contiguous instead of strided-gather; loaded in 256KB pieces so the
    # first matmul chain starts on the first piece.
    xf0 = nc.dram_tensor("xf0", [2, 128, 8, 512], BF16, kind="ExternalInput").ap()
    wqf = nc.dram_tensor("wqf", [2, 128, 8, 4 * HD], BF16, kind="ExternalInput").ap()
    out = nc.dram_tensor("out", [NSH, D], F32, kind="ExternalOutput").ap()
    a2a_in = [
        [
            nc.dram_tensor(f"a2a_in{b}_{h}", [NC, 128, 256], BF16).ap()
            for h in range(HPC)
        ]
        for b in range(B)
    ]
    a2a_out = [
        [
            nc.dram_tensor(f"a2a_out{b}_{h}", [NC, 128, 256], BF16).ap()
            for h in range(HPC)
        ]
        for b in range(B)
    ]

    MUL = mybir.AluOpType.mult
    ADD = mybir.AluOpType.add
    SUB = mybir.AluOpType.subtract
    EXP = mybir.ActivationFunctionType.Exp

    with tile.TileContext(nc, num_cores=NC) as tc:
        with (
            tc.tile_pool(name="const", bufs=1) as constp,
            tc.tile_pool(name="wqk", bufs=1) as wqkp,
            tc.tile_pool(name="wv", bufs=1) as wvp,
            tc.tile_pool(name="wo", bufs=1) as wop,
            tc.tile_pool(name="persist", bufs=1) as persist,
        ):
            wqk_sb = wqkp.tile([128, 16, 512], BF16, name="wqk_sb")
            wv_sb = wvp.tile([128, 16, 256], BF16, name="wv_sb")
            wo_sb = wop.tile([128, 16, D], BF16, name="wo_sb")
            tri_sb = constp.tile([128, 1024], BF16, name="tri_sb")
            ones_col = constp.tile([128, 1], BF16, name="ones_col")

            with (
                tc.tile_pool(name="xt", bufs=5) as xtp,
                tc.tile_pool(name="tab", bufs=2) as tabp,
                tc.tile_pool(name="rope", bufs=2) as ropep,
                tc.tile_pool(name="stage", bufs=3) as stagep,
                tc.tile_pool(name="pt", bufs=5) as ptp,
                tc.tile_pool(name="small", bufs=2) as smallp,
                tc.tile_pool(name="ots", bufs=3) as otsp,
                tc.tile_pool(name="rsc", bufs=2) as rscp,
                tc.tile_pool(name="bc", bufs=2) as bcp,
                tc.tile_pool(name="pst", bufs=2, space="PSUM") as pstp,
                tc.tile_pool(name="pov", bufs=2, space="PSUM") as povp,
                tc.tile_pool(name="psmall", bufs=2, space="PSUM") as psmallp,
            ):
                # startup loads, piecewise so the first matmul chain can
                # start as soon as its first 256KB lands.
                xh_first = []
                for half in range(2):
                    t = xtp.tile([128, 8, 512], BF16, tag="xt", name=f"xt_0_0_{half}")
                    for q in range(4):
                        # alternate queues so the x and wqk pieces transfer
                        # in parallel at startup
                        nc.sync.dma_start(
                            out=t[:, 2 * q : 2 * q + 2, :],
                            in_=xf0[half][:, 2 * q : 2 * q + 2, :],
                        )
                    xh_first.append(t)
                    for q in range(4):
                        nc.gpsimd.dma_start(
                            out=wqk_sb[:, 8 * half + 2 * q : 8 * half + 2 * q + 2, :],
                            in_=wqf[half][:, 2 * q : 2 * q + 2, :],
                        )
                nc.sync.dma_start(out=wv_sb[:, :, :], in_=wvf)
                nc.sync.dma_start(out=tri_sb[:, :], in_=tri01[:, :])
                nc.vector.memset(ones_col[:, :], 1.0)
                # first-use ptw tiles are partially read (then zeroed) by the
                # mask multiplies before exp ever wrote them: scrub SBUF
                # garbage (inf/nan bit patterns would make 0*x = nan).
                for q in range(5):
                    t = ptp.tile([128, 1024], BF16, tag="pt", name=f"ptz_{q}")
                    nc.vector.memset(t[:, :], 0.0)

                def load_x(b, j):
                    n0 = b * N + 512 * j
                    xh = []
                    for half in range(2):
                        t = xtp.tile(
                            [128, 8, 512], BF16, tag="xt", name=f"xt_{b}_{j}_{half}"
                        )
                        nc.sync.dma_start(
                            out=t[:, :, :],
                            in_=xT.rearrange("(k p) n -> p k n", p=128)[
                                :, 8 * half : 8 * half + 8, n0 : n0 + 512
                            ],
                        )
                        xh.append(t)
                    return xh

                def load_tabs(b, j):
                    n0 = b * N + 512 * j
                    tabt = []
                    for ti in range(4):
                        tt = tabp.tile([128, 512], BF16, tag=f"tab{ti}", name=f"tab{ti}_{b}_{j}")
                        nc.sync.dma_start(out=tt[:, :], in_=tabs[ti, :, n0 : n0 + 512])
                        tabt.append(tt)
                    return tabt

                def qk_pair(b, j, pair, xh, qh_sb, tabt):
                    pw = pstp.tile([128, 1024], F32, tag="pst", name=f"pw_{b}_{j}_{pair}")
                    psA = pw[:, 0:512]
                    psB = pw[:, 512:1024]
                    for mt, pst_ in ((pair, psA), (pair + 1, psB)):
                        for k in range(16):
                            nc.tensor.matmul(
                                pst_,
                                lhsT=(wqk_sb[:, k, 128 * mt : 128 * mt + 128]),
                                rhs=(xh[k // 8][:, k % 8, :]),
                                start=(k == 0),
                                stop=(k == 15),
                            )
                    ci = 0 if pair == 0 else 2
                    t1 = ropep.tile([128, 512], BF16, tag="t1", name=f"t1_{b}_{j}_{pair}")
                    t2 = ropep.tile([128, 512], BF16, tag="t2", name=f"t2_{b}_{j}_{pair}")
                    t3 = ropep.tile([128, 512], BF16, tag="t3", name=f"t3_{b}_{j}_{pair}")
                    t4 = ropep.tile([128, 512], BF16, tag="t4", name=f"t4_{b}_{j}_{pair}")
                    nc.vector.tensor_tensor(t1[:, :], psA, tabt[ci][:, :], MUL)
                    nc.vector.tensor_tensor(t2[:, :], psB, tabt[ci + 1][:, :], MUL)
                    nc.vector.tensor_tensor(t3[:, :], psB, tabt[ci][:, :], MUL)
                    nc.vector.tensor_tensor(t4[:, :], psA, tabt[ci + 1][:, :], MUL)
                    sl = stagep.tile([128, 512], BF16, tag="sl", name=f"sl_{b}_{j}_{pair}")
                    sh = stagep.tile([128, 512], BF16, tag="sh", name=f"sh_{b}_{j}_{pair}")
                    nc.vector.tensor_tensor(sl[:, :], t1[:, :], t2[:, :], SUB)
                    nc.vector.tensor_tensor(sh[:, :], t3[:, :], t4[:, :], ADD)
                    # repack: per-head [lo;hi] tiles for full-contract scores.
                    base = 0 if pair == 0 else 2
                    cs = slice(512 * j, 512 * (j + 1))
                    nc.sync.dma_start(out=qh_sb[0:64, base, cs], in_=sl[0:64, :])
                    nc.sync.dma_start(out=qh_sb[0:64, base + 1, cs], in_=sl[64:128, :])
                    nc.sync.dma_start(out=qh_sb[64:128, base, cs], in_=sh[0:64, :])
                    nc.sync.dma_start(out=qh_sb[64:128, base + 1, cs], in_=sh[64:128, :])

                def phase_a_chunk(b, j, qh_sb, vT_sb, xh, tabt):
                    for pair in (0, 2):
                        qk_pair(b, j, pair, xh, qh_sb, tabt)
                    for mt in range(4):
                        pv = povp.tile([128, 256], F32, tag="pov", name=f"psV_{b}_{j}_{mt}")
                        for k in range(16):
                            nc.tensor.matmul(
                                pv[:, :],
                                lhsT=(xh[k // 8][:, k % 8, 128 * mt : 128 * mt + 128]),
                                rhs=(wv_sb[:, k, :]),
                                start=(k == 0),
                                stop=(k == 15),
                            )
                        nc.scalar.copy(vT_sb[:, 4 * j + mt, :], pv[:, :])

                def phase_b_unit(b, h, j, qh_sb, vT_sb):
                    nt = 4 * j + 4
                    nslab = nt // 2
                    ov = povp.tile([128, 512], F32, tag="pov", name=f"ov_{b}_{h}_{j}")
                    rs_c = rscp.tile([128, 512], BF16, tag="rsc", name=f"rsc_{b}_{h}_{j}")
                    issued = 0
                    for m in range(nslab):
                        # the two diagonal slabs pack (higher, lower) key
                        # tiles so the valid region stays contiguous-ish.
                        if m == nslab - 2:
                            pair = (4 * j + 1, 4 * j + 0)
                            diag = True
                        elif m == nslab - 1:
                            pair = (4 * j + 3, 4 * j + 2)
                            diag = True
                        else:
                            pair = (2 * m, 2 * m + 1)
                            diag = False
                        pw = pstp.tile(
                            [128, 1024], F32, tag="pst", name=f"stw_{b}_{h}_{j}_{m}"
                        )
                        ptw = ptp.tile(
                            [128, 1024], BF16, tag="pt", name=f"pt_{b}_{h}_{j}_{m}"
                        )
                        for half, t in enumerate(pair):
                            if diag:
                                f0 = 128 * (t - 4 * j)
                            else:
                                f0 = 0
                            nc.tensor.matmul(
                                pw[:, 512 * half + f0 : 512 * half + 512],
                                lhsT=(qh_sb[:, 2 + h, 128 * t : 128 * t + 128]),
                                rhs=(qh_sb[:, h, 512 * j + f0 : 512 * (j + 1)]),
                                start=True,
                                stop=True,
                            )
                        if diag:
                            lo = 128 if m == nslab - 2 else 384
                            nc.scalar.activation(ptw[:, lo:1024], pw[:, lo:1024], EXP)
                        else:
                            nc.scalar.activation(ptw[:, :], pw[:, :], EXP)
                        if diag:
                            # zero the fully-masked cols + the causal band
                            for half, t in enumerate(pair):
                                f0 = 128 * (t - 4 * j)
                                w = f0 + 128
                                nc.vector.tensor_tensor(
                                    ptw[:, 512 * half : 512 * half + w],
                                    ptw[:, 512 * half : 512 * half + w],
                                    tri_sb[:, 512 - f0 : 512 - f0 + w],
                                    MUL,
                                )
                        # bf16 row-sum accumulation over key tiles
                        if m == 0:
                            nc.vector.tensor_tensor(
                                rs_c[:, :], ptw[:, 0:512], ptw[:, 512:1024], ADD
                            )
                        else:
                            nc.vector.tensor_tensor(
                                rs_c[:, :], rs_c[:, :], ptw[:, 0:512], ADD
                            )
                            nc.vector.tensor_tensor(
                                rs_c[:, :], rs_c[:, :], ptw[:, 512:1024], ADD
                            )
                        for half, t in enumerate(pair):
                            f0 = 128 * (t - 4 * j) if diag else 0
                            issued += 1
                            nc.tensor.matmul(
                                ov[:, f0:512],
                                lhsT=(vT_sb[:, t, 128 * h : 128 * h + 128]),
                                rhs=(ptw[:, 512 * half + f0 : 512 * half + 512]),
                                start=(issued == 1),
                                stop=(issued == nt),
                            )
                    rsum = psmallp.tile(
                        [1, 512], F32, tag="rsum", name=f"rsum_{b}_{h}_{j}"
                    )
                    nc.tensor.matmul(
                        rsum[:, :],
                        lhsT=ones_col[:, :],
                        rhs=rs_c[:, :],
                        start=True,
                        stop=True,
                    )
                    rinv = smallp.tile([1, 512], F32, tag="rinv", name=f"rinv_{b}_{h}_{j}")
                    nc.vector.reciprocal_approx_fast(rinv[:, :], rsum[:, :])
                    binv = bcp.tile([128, 512], F32, tag="binv", name=f"binv_{b}_{h}_{j}")
                    nc.gpsimd.partition_broadcast(binv[:, :], rinv[:, :])
                    ot = otsp.tile([128, 512], BF16, tag="ot", name=f"ot_{b}_{h}_{j}")
                    nc.vector.tensor_tensor(ot[:, :], ov[:, :], binv[:, :], MUL)
                    # a2a writes ride the (idle) gpsimd queue: their wait
                    # conditions must not block the sync queue's x prefetches.
                    nc.gpsimd.dma_start(
                        out=a2a_in[b][h][2 * j, :, :], in_=ot[:, 0:256]
                    )
                    nc.gpsimd.dma_start(
                        out=a2a_in[b][h][2 * j + 1, :, :], in_=ot[:, 256:512]
                    )

                def emit_cc(b, h):
                    nc.gpsimd.collective_compute(
                        "AllToAll",
                        mybir.AluOpType.bypass,
                        replica_groups=[list(range(NC))],
                        ins=[a2a_in[b][h].opt()],
                        outs=[a2a_out[b][h].opt()],
                    )

                # chunk (0,0) inputs are the startup piece loads
                pre_x = {(0, 0): xh_first}
                pre_tab = {(0, 0): load_tabs(0, 0)}
                for b in range(B):
                    qh_sb = persist.tile(
                        [128, 4, N], BF16, tag="qh", name=f"qh_b{b}"
                    )
                    vT_sb = persist.tile(
                        [128, 16, HPC * HD], BF16, tag="vT", name=f"vT_b{b}"
                    )
                    # projection chunks interleaved with the attention units
                    # they unblock: unit (h, j) needs only chunks 0..j.
                    # x/tabs for the NEXT chunk are prefetched first each
                    # iteration so no compute-gated DMA sits ahead of them
                    # in the sync queue.
                    for j in range(4):
                        nxt = (b, j + 1) if j < 3 else (b + 1, 0)
                        if nxt[0] < B:
                            pre_x[nxt] = load_x(*nxt)
                            pre_tab[nxt] = load_tabs(*nxt)
                        phase_a_chunk(
                            b, j, qh_sb, vT_sb,
                            pre_x.pop((b, j)), pre_tab.pop((b, j)),
                        )
                        for h in range(HPC):
                            phase_b_unit(b, h, j, qh_sb, vT_sb)
                            # each head's collective fires as soon as its
                            # last unit is done
                            if j == 3:
                                emit_cc(b, h)
                        if b == 0:
                            # w_o cache fill rides under batch-0 compute in
                            # 512KB pieces: one 8MB transfer would monopolize
                            # the DMA engines for ~23us and starve the
                            # latency-critical qh repack transfers behind it.
                            for k in range(4 * j, 4 * j + 4):
                                nc.sync.dma_start(
                                    out=wo_sb[:, k, :], in_=wof[:, k, :]
                                )

            # ---------------- phase C: o_proj ------------------------------
            # opin k-tile order is (h, src) -> wof rows are host-permuted to
            # match.  All half-1 chains split at the k=8 boundary: the low
            # halves depend only on the (b1,h0) collective, so they fill the
            # window while the last collective completes.
            with (
                tc.tile_pool(name="opin", bufs=1) as opinp,
                tc.tile_pool(name="outs", bufs=4) as outsp,
                tc.tile_pool(name="pc", bufs=8, space="PSUM") as pcp,
            ):
                opins = []
                for half in range(B):
                    opin = opinp.tile([128, 16, 256], BF16, tag=f"opin{half}", name=f"opin{half}")
                    for h in range(HPC):
                        nc.sync.dma_start(
                            out=opin[:, 8 * h : 8 * h + 8, :],
                            in_=a2a_out[half][h].rearrange("r p n -> p r n"),
                        )
                    opins.append(opin)

                def c_chain(half, ns, dc, pc, k0, k1):
                    opin = opins[half]
                    for k in range(k0, k1):
                        nc.tensor.matmul(
                            pc[:, :],
                            lhsT=(opin[:, k, 128 * ns : 128 * ns + 128]),
                            rhs=(wo_sb[:, k, 512 * dc : 512 * (dc + 1)]),
                            start=(k == 0),
                            stop=(k == 15),
                        )

                def c_finish(half, ns, dc, pc, ost):
                    nc.scalar.copy(ost[:, 512 * dc : 512 * (dc + 1)], pc[:, :])
                    r0 = 256 * half + 128 * ns
                    # out DMAs on the scalar queue: the sync queue's head is
                    # the opin[1] loads waiting on the last collectives.
                    nc.scalar.dma_start(
                        out=out[r0 : r0 + 128, 512 * dc : 512 * (dc + 1)],
                        in_=ost[:, 512 * dc : 512 * (dc + 1)],
                    )

                osts = {}
                for ns in range(2):
                    osts[0, ns] = outsp.tile([128, D], F32, tag="outs", name=f"os_0_{ns}")
                    for dc in range(4):
                        pc = pcp.tile([128, 512], F32, tag="pc", name=f"pc_0_{dc}_{ns}")
                        c_chain(0, ns, dc, pc, 0, 16)
                        c_finish(0, ns, dc, pc, osts[0, ns])
                pcs = {}
                for ns in range(2):
                    osts[1, ns] = outsp.tile([128, D], F32, tag="outs", name=f"os_1_{ns}")
                    for dc in range(4):
                        pcs[ns, dc] = pcp.tile([128, 512], F32, tag="pc", name=f"pc_1_{dc}_{ns}")
                        c_chain(1, ns, dc, pcs[ns, dc], 0, 8)
                for ns in range(2):
                    for dc in range(4):
                        c_chain(1, ns, dc, pcs[ns, dc], 8, 16)
                        c_finish(1, ns, dc, pcs[ns, dc], osts[1, ns])
    nc.compile()
    return nc


def _host_prep(x, w_qkv, w_o):
    bf = ml_dtypes.bfloat16
    xT = np.ascontiguousarray(x.reshape(BN, D).T).astype(bf)
    # o_proj k-tile order on device is (h_local, src_core): head g lives at
    # slot 8*(g % 2) + g // 2.  wof is the [p, k, d] partition-major pack.
    woT_n = np.asarray(w_o).T.reshape(H, HD, D)
    perm = [2 * s + hl for hl in range(HPC) for s in range(NC)]
    woT = np.ascontiguousarray(woT_n[perm].reshape(INNER, D)).astype(bf)
    wof = np.ascontiguousarray(woT.reshape(16, 128, D).transpose(1, 0, 2))

    inv_freq = 1.0 / (ROPE_BASE ** (np.arange(0, HD, 2, dtype=np.float32) / HD))
    ang = np.arange(N, dtype=np.float32)[:, None] * inv_freq[None, :]
    cos_h = np.cos(ang).T.astype(np.float32)      # [64, N]
    sin_h = np.sin(ang).T.astype(np.float32)      # [64, N] (magnitude)
    # duplicated for the two heads packed per 128-row block
    cos2 = np.concatenate([cos_h, cos_h], axis=0)  # [128, N]
    sin2 = np.concatenate([sin_h, sin_h], axis=0)
    cos_f = np.tile(cos2, (1, B))
    sin_f = np.tile(sin2, (1, B))
    scale = np.float32(1.0 / np.sqrt(HD))
    tabs = np.ascontiguousarray(
        np.stack([cos_f * scale, sin_f * scale, cos_f, sin_f], axis=0)
    ).astype(bf)

    xf0 = np.ascontiguousarray(
        xT.reshape(16, 128, BN)[:, :, 0:512]
        .reshape(2, 8, 128, 512)
        .transpose(0, 2, 1, 3)
    )

    p = np.arange(128)[:, None]
    c = np.arange(1024)[None, :]
    # multiplicative causal mask: 1 where visible, 0 where masked
    tri01 = np.where(p <= c - 512, 1.0, 0.0).astype(bf)

    in_maps = []
    for core in range(NC):
        h0 = core * HPC
        rq = slice(h0 * HD, (h0 + HPC) * HD)
        rk = slice(INNER + h0 * HD, INNER + (h0 + HPC) * HD)
        rv = slice(2 * INNER + h0 * HD, 2 * INNER + (h0 + HPC) * HD)
        wq = w_qkv[rq].reshape(HPC, HD, D)
        wk = w_qkv[rk].reshape(HPC, HD, D)
        # row order per block: [h0_lo, h1_lo | h0_hi, h1_hi] for q then k
        wqkT = np.ascontiguousarray(
            np.concatenate(
                [wq[0, :64], wq[1, :64], wq[0, 64:], wq[1, 64:],
                 wk[0, :64], wk[1, :64], wk[0, 64:], wk[1, 64:]], axis=0
            ).T
        ).astype(bf)
        wvT = np.ascontiguousarray(w_qkv[rv].T).astype(bf)
        wvfc = np.ascontiguousarray(wvT.reshape(16, 128, HPC * HD).transpose(1, 0, 2))
        wqf = np.ascontiguousarray(
            wqkT.reshape(2, 8, 128, 4 * HD).transpose(0, 2, 1, 3)
        )
        in_maps.append(
            dict(xT=xT, wqkT=wqkT, wvf=wvfc, wof=wof, tabs=tabs, tri01=tri01,
                 xf0=xf0, wqf=wqf)
        )
    return in_maps


def kernel(x, w_qkv, w_o, n_heads=None, head_dim=None, trace=False):
    global LAST_EXEC_NS, LAST_RESULTS
    x = np.asarray(x, dtype=np.float32)
    w_qkv = np.asarray(w_qkv, dtype=np.float32)
    w_o = np.asarray(w_o, dtype=np.float32)

    if "nc" not in _CACHE:
        _CACHE["nc"] = _build_program()
    nc = _CACHE["nc"]

    in_maps = _host_prep(x, w_qkv, w_o)
    res = None
    last_exc = None
    for attempt in range(4):
        try:
            res = bass_utils.run_bass_kernel_spmd(
                nc, in_maps, core_ids=list(range(NC)), trace=trace
            )
            break
        except Exception as e:  # transient compile_and_load / exec flakiness
            last_exc = e
            print(f"kernel attempt {attempt} failed: {e}", file=sys.stderr)
            time.sleep(5)
    if res is None:
        raise last_exc
    LAST_EXEC_NS = res.exec_time_ns
    LAST_RESULTS = res
    # core c returns [512, D]: rows 0:256 = batch0 rows 256c:256c+256,
    # rows 256:512 = batch1 rows 256c:256c+256.
    full = np.empty((B, N, D), dtype=np.float32)
    for c in range(NC):
        shard = res.results[c]["out"]
        full[0, 256 * c : 256 * c + 256] = shard[0:256]
        full[1, 256 * c : 256 * c + 256] = shard[256:512]
    return full
